# revision 16
# baseline (speedup 1.0000x reference)
"""Luong-attention GRU decoder step on 8 Trainium2 NeuronCores.

Sharding: data-parallel over batch for the attention streams (8 batches/core,
each batch's [2048,1024] encoder slice is read exactly once, flash-style
unnormalized softmax with a fixed -50 shift), vocab-parallel for the output
projection (out_w row-sharded 6250 rows/core), one AllGather of the concat
input [h_new | context] between the two phases.

Weights are passed pre-transposed (contraction dim on partitions) with biases
folded in as an extra ones-row of the contraction, so bias adds are free.
"""
import numpy as np

import concourse.bass as bass
import concourse.tile as tile
from concourse import mybir
from concourse.bass_utils import run_bass_kernel_spmd
from concourse.masks import make_identity
from concourse.vector_clock import ScopedClock, VectorClock

V = 50000
H = 1024
B = 64
S = 2048
NCORES = 8
BL = B // NCORES          # 8 local batches per core
VS = V // NCORES          # 6250 vocab rows per core
F32 = mybir.dt.float32

KA = H + 128              # 1152: GRU x-side contraction with bias row
KC = 2 * H + 128          # 2176: concat contraction with bias row
SCHUNK = 512              # s-values per enc tile (4 subchunks of 128)
NSH = S // SCHUNK         # 4 enc tiles per batch
WCH = 256                 # column chunk for streamed GRU/concat weights
LCH = 512                 # vocab column chunk for logits


class _TileContextFix(tile.TileContext):
    """This container's walrus codegen allows only one sync wait per
    instruction; split multi-wait instructions by hoisting extra waits onto
    preceding same-engine NoOps, and emit the kernel-tail drain as one drain
    per outstanding proc instead of a single multi-wait drain."""

    def __exit__(self, *args):
        ret = super().__exit__(*args)
        import bass_rust
        nc = self.nc

        def make_wait_carrier(engine, wait):
            # Emit a real drain via the engine API (appends to the current
            # bb tail), then pop it off so we can reinsert it where needed.
            d = nc.engines[engine].drain(fusable=False).ins
            for f in nc.m.functions:
                for bb in f.blocks:
                    il = bb.instructions
                    if il and il[-1] is d:
                        il.pop()
                        d.sync_info = bass_rust.SyncInfo(
                            on_wait=[wait], on_update=[])
                        return d
            raise RuntimeError("could not locate emitted drain")

        for f in nc.m.functions:
            for bb in f.blocks:
                insts = bb.instructions
                if not any(
                    i.sync_info is not None and len(i.sync_info.on_wait) > 1
                    for i in insts
                ):
                    continue
                new = []
                for inst in insts:
                    si = inst.sync_info
                    if si is not None and len(si.on_wait) > 1:
                        waits = list(si.on_wait)
                        for w in waits[:-1]:
                            new.append(make_wait_carrier(inst.engine, w))
                        inst.sync_info = bass_rust.SyncInfo(
                            on_wait=[waits[-1]], on_update=list(si.on_update))
                    new.append(inst)
                bb.instructions[:] = new
        return ret

    def _drain_and_barrier(self, tick_clock, wait_clock):
        gc = tick_clock.global_clock
        n = len(gc)
        for i in range(n):
            t = gc[i]
            if t > 0:
                vc = VectorClock([t if j == i else 0 for j in range(n)])
                d = self.nc.sync.drain()
                wait_clock.add_sem_waits(d.ins, ScopedClock({None: vc}))
        self.nc.all_engine_barrier()
        assert self.sems is not None
        popped = self.nc._tile_sem_poison_stack.pop()
        assert popped is self._sem_poison
        self.nc.clear_and_free_semaphores(list(self.sems.allocated().values()))
        self.nc.all_engine_barrier()


def _build():
    nc = bass.Bass(num_devices=NCORES)

    xT = nc.dram_tensor("xT", [KA, B], F32, kind="ExternalInput")
    hT = nc.dram_tensor("hT", [H, B], F32, kind="ExternalInput")
    h_nat = nc.dram_tensor("h_nat", [B, H], F32, kind="ExternalInput")
    w_ihT = nc.dram_tensor("w_ihT", [KA, 3 * H], F32, kind="ExternalInput")
    w_hhT = nc.dram_tensor("w_hhT", [H, 3 * H], F32, kind="ExternalInput")
    enc = nc.dram_tensor("enc", [BL, S, H], F32, kind="ExternalInput")
    cwT = nc.dram_tensor("cwT", [KC, H], F32, kind="ExternalInput")
    owT = nc.dram_tensor("owT", [KA, VS], F32, kind="ExternalInput")
    selT = nc.dram_tensor("selT", [B, BL], F32, kind="ExternalInput")
    selRep = nc.dram_tensor("selRep", [B, BL, 128], F32, kind="ExternalInput")

    logits = nc.dram_tensor("logits", [B, VS], F32, kind="ExternalOutput")
    hidden = nc.dram_tensor("hidden", [B, H], F32, kind="ExternalOutput")
    attnw = nc.dram_tensor("attnw", [BL, S], F32, kind="ExternalOutput")

    with _TileContextFix(nc) as tc:
        with (
            tc.tile_pool(name="const", bufs=1) as constp,
            tc.tile_pool(name="small", bufs=1) as smallp,
            tc.tile_pool(name="wstream", bufs=2) as wsp,
            tc.tile_pool(name="whstream", bufs=2) as whp,
            tc.tile_pool(name="encp", bufs=3) as encp,
            tc.tile_pool(name="scratch", bufs=2) as scr,
            tc.tile_pool(name="scratch1", bufs=1) as scr1,
            tc.tile_pool(name="outs", bufs=2) as outp,
            tc.tile_pool(name="psmm", bufs=2, space="PSUM") as psmm,
            tc.tile_pool(name="pssm", bufs=2, space="PSUM") as pssm,
            tc.tile_pool(name="psctx", bufs=2, space="PSUM") as psctx,
            tc.tile_pool(name="dram", bufs=1, space="DRAM") as dramp,
        ):
            # ---- constants ----
            identity = constp.tile([128, 128], F32)
            make_identity(nc, identity)
            ones_r = constp.tile([1, 128], F32)   # lhsT for partition-broadcast
            nc.vector.memset(ones_r, 1.0)
            ones_c = constp.tile([128, 1], F32)   # rhs for cross-partition sum
            nc.vector.memset(ones_c, 1.0)
            neg50 = constp.tile([128, 1], F32)    # softmax shift for exp
            nc.vector.memset(neg50, -50.0)

            # ---- static SBUF inputs ----
            xT_sb = smallp.tile([128, KA // 128, B], F32, tag="xT")
            nc.sync.dma_start(out=xT_sb, in_=xT.rearrange("(t p) b -> p t b", p=128))
            hT_sb = smallp.tile([128, H // 128, B], F32, tag="hT")
            nc.sync.dma_start(out=hT_sb, in_=hT.rearrange("(t p) b -> p t b", p=128))
            h_nat_sb = smallp.tile([B, H], F32, tag="h_nat")
            nc.sync.dma_start(out=h_nat_sb, in_=h_nat[:, :])
            selT_sb = smallp.tile([B, BL], F32, tag="selT")
            nc.sync.dma_start(out=selT_sb, in_=selT[:, :])
            selrep_sb = smallp.tile([B, BL, 128], F32, tag="selRep")
            nc.sync.dma_start(out=selrep_sb, in_=selRep[:, :, :])

            # ---- GRU: gi+gh fused in PSUM, gate math on DVE/ACT ----
            r_sb = smallp.tile([B, H], F32, tag="r")
            z_sb = smallp.tile([B, H], F32, tag="z")
            h_new = smallp.tile([B, H], F32, tag="h_new")
            nch = (3 * H) // WCH  # 12
            for c in range(nch):
                col0 = c * WCH
                wi = wsp.tile([128, KA // 128, WCH], F32, tag="wstream")
                nc.sync.dma_start(
                    out=wi,
                    in_=w_ihT[:, col0 : col0 + WCH].rearrange("(t p) n -> p t n", p=128),
                )
                wh = whp.tile([128, H // 128, WCH], F32, tag="whstream")
                nc.sync.dma_start(
                    out=wh,
                    in_=w_hhT[:, col0 : col0 + WCH].rearrange("(t p) n -> p t n", p=128),
                )
                if c < 8:
                    # r (c<4) and z (4<=c<8): gi+gh accumulate in one psum
                    ps = psmm.tile([B, WCH], F32, tag="mm")
                    for t in range(KA // 128):
                        nc.tensor.matmul(ps, xT_sb[:, t, :], wi[:, t, :],
                                         start=(t == 0), stop=False)
                    for t in range(H // 128):
                        nc.tensor.matmul(ps, hT_sb[:, t, :], wh[:, t, :],
                                         start=False, stop=(t == H // 128 - 1))
                    dst = r_sb if c < 4 else z_sb
                    off = col0 if c < 4 else col0 - H
                    nc.scalar.activation(
                        out=dst[:, off : off + WCH], in_=ps,
                        func=mybir.ActivationFunctionType.Sigmoid,
                    )
                else:
                    # n gate: need gi and gh separately (n = tanh(gi + r*gh))
                    off = col0 - 2 * H
                    ps_i = psmm.tile([B, WCH], F32, tag="mm")
                    for t in range(KA // 128):
                        nc.tensor.matmul(ps_i, xT_sb[:, t, :], wi[:, t, :],
                                         start=(t == 0), stop=(t == KA // 128 - 1))
                    ps_h = psmm.tile([B, WCH], F32, tag="mm")
                    for t in range(H // 128):
                        nc.tensor.matmul(ps_h, hT_sb[:, t, :], wh[:, t, :],
                                         start=(t == 0), stop=(t == H // 128 - 1))
                    tmp = scr.tile([B, WCH], F32, tag="gru_tmp")
                    nc.vector.tensor_mul(tmp, r_sb[:, off : off + WCH], ps_h)
                    nc.vector.tensor_add(tmp, tmp, ps_i)
                    n_c = scr.tile([B, WCH], F32, tag="gru_n")
                    nc.scalar.activation(out=n_c, in_=tmp,
                                         func=mybir.ActivationFunctionType.Tanh)
                    # h_new = n + z*(h - n)
                    d_c = scr.tile([B, WCH], F32, tag="gru_d")
                    nc.vector.tensor_sub(d_c, h_nat_sb[:, off : off + WCH], n_c)
                    nc.vector.tensor_mul(d_c, z_sb[:, off : off + WCH], d_c)
                    nc.vector.tensor_add(h_new[:, off : off + WCH], n_c, d_c)

            nc.sync.dma_start(out=hidden[:, :], in_=h_new)

            # local batch rows of h_new via selection matmul (avoids per-core
            # dynamic indexing): h_loc = selT.T @ h_new
            h_loc = smallp.tile([BL, H], F32, tag="h_loc")
            for half in range(2):
                ps = psmm.tile([BL, 512], F32, tag="mm")
                nc.tensor.matmul(ps, selT_sb, h_new[:, half * 512 : half * 512 + 512],
                                 start=True, stop=True)
                nc.vector.tensor_copy(h_loc[:, half * 512 : half * 512 + 512], ps)

            # ---- attention: one pass per local batch over its enc slice ----
            cin_loc = smallp.tile([BL, 2 * H], F32, tag="cin_loc")
            nc.vector.tensor_copy(cin_loc[:, 0:H], h_loc)
            for lb in range(BL):
                # replicate this batch's h_new row across 128 partitions:
                # h_rep = selRep[:, lb].T @ h_new  (select + broadcast in one op)
                h_rep = scr.tile([128, H], F32, tag="h_rep")
                for half in range(2):
                    psr = pssm.tile([128, 512], F32, tag="sm")
                    nc.tensor.matmul(
                        psr, selrep_sb[:, lb, :],
                        h_new[:, half * 512 : half * 512 + 512],
                        start=True, stop=True)
                    nc.vector.tensor_copy(h_rep[:, half * 512 : half * 512 + 512], psr)

                scores = scr.tile([128, S // 128], F32, tag="scores")
                probs = scr.tile([128, S // 128], F32, tag="probs")
                ctx_ps = psctx.tile([1, H], F32, tag="ctx")
                prod = scr1.tile([128, H], F32, tag="prod")
                for ti in range(NSH):
                    et = encp.tile([128, SCHUNK // 128, H], F32, tag="enc")
                    nc.sync.dma_start(
                        out=et,
                        in_=enc[lb, ti * SCHUNK : (ti + 1) * SCHUNK, :].rearrange(
                            "(u p) h -> p u h", p=128),
                    )
                    for u in range(SCHUNK // 128):
                        uu = ti * (SCHUNK // 128) + u
                        nc.vector.tensor_mul(prod, et[:, u, :], h_rep)
                        nc.vector.reduce_sum(scores[:, uu : uu + 1], prod,
                                             axis=mybir.AxisListType.X)
                        # unnormalized flash: exp(score - 50), shift-invariant
                        nc.scalar.activation(
                            out=probs[:, uu : uu + 1], in_=scores[:, uu : uu + 1],
                            func=mybir.ActivationFunctionType.Exp, bias=neg50[:, 0:1],
                        )
                        for half in range(2):
                            nc.tensor.matmul(
                                ctx_ps[:, half * 512 : half * 512 + 512],
                                probs[:, uu : uu + 1],
                                et[:, u, half * 512 : half * 512 + 512],
                                start=(uu == 0), stop=(uu == S // 128 - 1),
                            )

                # normalize: total = sum over all partitions+chunks of probs
                sumexp = scr.tile([128, 1], F32, tag="sumexp")
                nc.vector.reduce_sum(sumexp, probs, axis=mybir.AxisListType.X)
                tot_ps = pssm.tile([1, 1], F32, tag="sm")
                nc.tensor.matmul(tot_ps, sumexp, ones_c, start=True, stop=True)
                recip = scr.tile([1, 1], F32, tag="recip")
                nc.vector.reciprocal(recip, tot_ps)
                rr_ps = pssm.tile([128, 1], F32, tag="sm")
                nc.tensor.matmul(rr_ps, ones_r, recip, start=True, stop=True)
                rr = scr.tile([128, 1], F32, tag="rr")
                nc.vector.tensor_copy(rr, rr_ps)

                attn_n = scr.tile([128, S // 128], F32, tag="attn_n")
                nc.vector.tensor_scalar_mul(attn_n, probs, rr)
                at_ps = pssm.tile([S // 128, 128], F32, tag="sm")
                nc.tensor.transpose(at_ps, attn_n, identity)
                at_sb = outp.tile([S // 128, 128], F32, tag="at_sb")
                nc.vector.tensor_copy(at_sb, at_ps)
                nc.sync.dma_start(
                    out=attnw[lb, :].rearrange("(t q) -> t q", q=128), in_=at_sb)

                # context row: normalize at partition 0, then DMA into this
                # batch's cin_loc row (SBUF->SBUF DMA shifts partitions)
                ctx_row = scr.tile([1, H], F32, tag="ctx_row")
                nc.vector.tensor_scalar_mul(ctx_row, ctx_ps, recip)
                nc.sync.dma_start(out=cin_loc[lb : lb + 1, H : 2 * H], in_=ctx_row)

            # ---- AllGather cin across the 8 cores ----
            cc_in = dramp.tile([BL, 2 * H], F32)
            cc_out = dramp.tile([B, 2 * H], F32)
            nc.sync.dma_start(out=cc_in, in_=cin_loc)
            nc.gpsimd.collective_compute(
                "AllGather", mybir.AluOpType.bypass,
                replica_groups=[list(range(NCORES))],
                ins=[cc_in[:, :].opt()], outs=[cc_out[:, :].opt()],
            )
            cin_all = smallp.tile([B, 2 * H], F32, tag="cin_all")
            nc.sync.dma_start(out=cin_all, in_=cc_out[:, :])

            # cinT [2H+128, B] tiles with trailing ones-row for concat bias
            cinT = smallp.tile([128, KC // 128, B], F32, tag="cinT")
            nc.vector.memset(cinT[:, KC // 128 - 1, :], 0.0)
            nc.vector.memset(cinT[0:1, KC // 128 - 1, :], 1.0)
            for t in range(2 * H // 128):
                tp = pssm.tile([128, B], F32, tag="sm")
                nc.tensor.transpose(tp, cin_all[:, t * 128 : (t + 1) * 128],
                                    identity[0:B, 0:B])
                nc.vector.tensor_copy(cinT[:, t, :], tp)

            # ---- concat layer: co = tanh(cin @ concat_w.T + b) ----
            co_sb = smallp.tile([B, H], F32, tag="co")
            for c in range(H // WCH):
                col0 = c * WCH
                cw = wsp.tile([128, KC // 128, WCH], F32, tag="wstream")
                nc.sync.dma_start(
                    out=cw,
                    in_=cwT[:, col0 : col0 + WCH].rearrange("(t p) n -> p t n", p=128),
                )
                ps = psmm.tile([B, WCH], F32, tag="mm")
                for t in range(KC // 128):
                    nc.tensor.matmul(ps, cinT[:, t, :], cw[:, t, :],
                                     start=(t == 0), stop=(t == KC // 128 - 1))
                nc.scalar.activation(out=co_sb[:, col0 : col0 + WCH], in_=ps,
                                     func=mybir.ActivationFunctionType.Tanh)

            # coT [H+128, B] with ones-row for out_b
            coT = smallp.tile([128, KA // 128, B], F32, tag="coT")
            nc.vector.memset(coT[:, KA // 128 - 1, :], 0.0)
            nc.vector.memset(coT[0:1, KA // 128 - 1, :], 1.0)
            for t in range(H // 128):
                tp = pssm.tile([128, B], F32, tag="sm")
                nc.tensor.transpose(tp, co_sb[:, t * 128 : (t + 1) * 128],
                                    identity[0:B, 0:B])
                nc.vector.tensor_copy(coT[:, t, :], tp)

            # ---- vocab projection (V-sharded): logits = co @ out_w.T + b ----
            nvc = (VS + LCH - 1) // LCH
            for c in range(nvc):
                col0 = c * LCH
                w = min(LCH, VS - col0)
                ow = wsp.tile([128, KA // 128, LCH], F32, tag="wstream")
                nc.sync.dma_start(
                    out=ow[:, :, :w],
                    in_=owT[:, col0 : col0 + w].rearrange("(t p) n -> p t n", p=128),
                )
                ps = psmm.tile([B, LCH], F32, tag="mm")
                for t in range(KA // 128):
                    nc.tensor.matmul(ps[:, :w], coT[:, t, :], ow[:, t, :w],
                                     start=(t == 0), stop=(t == KA // 128 - 1))
                lg = outp.tile([B, LCH], F32, tag="lg")
                nc.vector.tensor_copy(lg[:, :w], ps[:, :w])
                nc.sync.dma_start(out=logits[:, col0 : col0 + w], in_=lg[:, :w])

    return nc


_built = None


def _get_nc():
    global _built
    if _built is None:
        _built = _build()
    return _built


def prepare_inputs(input_seq, last_hidden, encoder_outputs, embedding,
                   w_ih, w_hh, b_ih, b_hh, concat_w, concat_b, out_w, out_b):
    """Host-side sharding/layout: returns in_maps for the 8 cores."""
    f = np.float32
    input_seq = np.asarray(input_seq)
    x = np.asarray(embedding)[input_seq].astype(f)          # [B, H]
    h = np.asarray(last_hidden)[0].astype(f)                # [B, H]

    xT = np.zeros((KA, B), f)
    xT[:H] = x.T
    xT[H] = 1.0
    hT = np.ascontiguousarray(h.T)
    w_ihT = np.zeros((KA, 3 * H), f)
    w_ihT[:H] = np.asarray(w_ih).T
    w_ihT[H] = np.asarray(b_ih).astype(f) + np.asarray(b_hh).astype(f)
    w_hhT = np.ascontiguousarray(np.asarray(w_hh).T.astype(f))
    cwT = np.zeros((KC, H), f)
    cwT[: 2 * H] = np.asarray(concat_w).T
    cwT[2 * H] = np.asarray(concat_b).astype(f)
    ow = np.asarray(out_w)
    ob = np.asarray(out_b)
    enc_all = np.asarray(encoder_outputs)

    in_maps = []
    for k in range(NCORES):
        b0 = k * BL
        v0 = k * VS
        owT = np.zeros((KA, VS), f)
        owT[:H] = ow[v0 : v0 + VS].T
        owT[H] = ob[v0 : v0 + VS]
        selT = np.zeros((B, BL), f)
        selRep = np.zeros((B, BL, 128), f)
        for j in range(BL):
            selT[b0 + j, j] = 1.0
            selRep[b0 + j, j, :] = 1.0
        in_maps.append({
            "xT": xT, "hT": hT, "h_nat": h, "w_ihT": w_ihT, "w_hhT": w_hhT,
            "enc": np.ascontiguousarray(
                enc_all[:, b0 : b0 + BL, :].transpose(1, 0, 2)).astype(f),
            "cwT": cwT, "owT": owT, "selT": selT, "selRep": selRep,
        })
    return in_maps


def assemble_outputs(results):
    logits = np.concatenate([r["logits"] for r in results], axis=1)
    hidden = results[0]["hidden"][None]
    attnw = np.concatenate([r["attnw"] for r in results], axis=0)[:, None, :]
    return logits, hidden, attnw


def kernel(**inputs):
    in_maps = prepare_inputs(**inputs)
    nc = _get_nc()
    res = run_bass_kernel_spmd(nc, in_maps, core_ids=list(range(NCORES)),
                               trace=False)
    return assemble_outputs(res.results)
